# revision 65
# baseline (speedup 1.0000x reference)
"""AdaptiveMixing Trainium2 kernel (8 NeuronCores, pure data parallel).

Math: out[b,s] = sum_k softmax(ada_mask[b,s])[k] * xpad[b, s+k-10]  (K=21)

Key idea: with S=128 on SBUF partitions and H*W on the free dim, the
spectral sliding-window reduction is a single 128x128 banded matmul
per free-dim tile:
    out[s_o, f] = sum_{s} Wb[s_o, s] * x[s, f],
    Wb[s_o, s] = w[s_o, s - s_o + 10] for |s - s_o| <= 10 else 0
so the TensorEngine does all cross-partition movement:
    out = lhsT.T @ x with lhsT[s, s_o] = Wb[s_o, s].

Band build (on device, off the DMA critical path):
  1. dstack[p,k,f] = 1 if f == p + k - 10 else 0   (gpsimd affine_select,
     no input dependency -- runs at kernel start)
  2. softmax numerator wexp = exp(mask - max) (+ row sums via accum_out);
     normalization is folded into the PSUM->SBUF epilogue as a
     per-partition reciprocal multiply.
  3. DW_k = dstack[:,k,:] * wexp[:,k]              (21 small DVE muls)
  4. band = sum_k DW_k.T                           (21 PSUM-accumulated PE
     matmuls against the identity; pipelines behind the DVE muls)

The x/out HBM streams and all matmul operands are bf16 (the kernel is
memory-bound; halving the stream bytes halves the DMA time). Softmax,
PSUM accumulation and the per-partition normalization stay f32. Costs
~5e-3 absmax rel err vs the f32 reference (gate: 2e-2). KERNEL_DT=f32r
or f32 restore 4-byte streams (~1.5e-4 / ~3e-7 err, ~33.3/38us).

Sharding (host side): core i <- batch b = i//2, H-half h = i%2.
Each core handles x[b, :, h*64:(h+1)*64, :] as a (128, 8192) slab.
No communication needed.
"""

import os

import numpy as np

B, S, H, W = 4, 128, 128, 128
K = 21
PAD = 10
N_CORES = 8
H_SPLIT = 2
HS = H // H_SPLIT          # 64 rows of H per core
FREE = HS * W              # 8192
CHUNK = 2048               # free-dim elements per DMA chunk
MM_N = 512                 # matmul free dim per instruction

# Stream dtype: bf16 halves HBM traffic (the kernel is memory-bound) and
# runs the PE at full rate; PSUM accumulation and the softmax stay f32.
# KERNEL_DT=f32r / f32 fall back to 4-byte streams for A/B testing.
KERNEL_DT = os.environ.get("KERNEL_DT", "bf16")

_COMPILED = {}


def _install_light_tail():
    """Tile's stock tail is drain + all-engine barrier + sem clears +
    barrier (~4-7us). Replace both barriers with a single gpsimd wait-fence
    on the same global clock: the gpsimd-issued dma/sem clears only need
    every proc's final tick, and NEFF completion already requires every
    queue (including gpsimd's clears) to finish."""
    import concourse.tile as tile

    if getattr(tile.TileContext, "_light_tail", False):
        return

    def _drain_and_barrier(self, tick_clock, wait_clock):
        drain_inst = self.nc.sync.drain()
        wait_clock.add_sem_waits(
            drain_inst.ins,
            _scoped_clock({None: tick_clock.global_clock}),
        )
        fence = self.nc.gpsimd.nop(nofuse=True, hint="tail_fence")
        wait_clock.add_sem_waits(
            fence.ins,
            _scoped_clock({None: tick_clock.global_clock}),
        )
        assert self.sems is not None
        popped = self.nc._tile_sem_poison_stack.pop()
        assert popped is self._sem_poison
        self.nc.clear_and_free_semaphores(list(self.sems.allocated().values()))

    import bass_rust as _bass_rust

    def _scoped_clock(d):
        return _bass_rust.ScopedClock(d)

    tile.TileContext._drain_and_barrier = _drain_and_barrier
    tile.TileContext._light_tail = True


def _hoist_preamble(nc, hoist):
    """Move wait-free setup instructions (input DMA issues, dstack memset,
    act-table load) from the tile body into each engine's pre-barrier slot in
    the entry block. Engine boot skew makes the entry barrier take ~3us; work
    issued before an engine's barrier EVSEM runs inside that window for free,
    so input data is already streaming while the NEFF boots."""
    import concourse.mybir as mybir

    f = nc.m.functions[0]
    entry = f.blocks[0]
    body = f.blocks[1]

    eng_of = {
        "SP": mybir.EngineType.SP,
        "Pool": mybir.EngineType.Pool,
        "Activation": mybir.EngineType.Activation,
    }

    hoist = dict(hoist)

    for eng_key, names in hoist.items():
        eng = eng_of[eng_key]
        name_set = set(names)
        moved = []
        keep = []
        for ins in body.instructions:
            if ins.name in name_set:
                si = ins.sync_info
                if si is not None and si.on_wait:
                    keep.append(ins)  # not wait-free; leave in place
                else:
                    moved.append(ins)
            else:
                keep.append(ins)
        if not moved:
            continue
        body.instructions[:] = keep
        # insert before this engine's barrier EVSEM in the entry block
        idx = None
        for i, ins in enumerate(entry.instructions):
            if (
                type(ins).__name__ == "InstEventSemaphore"
                and ins.engine == eng
            ):
                idx = i
                break
        assert idx is not None, f"no entry barrier EVSEM for {eng_key}"
        for j, ins in enumerate(moved):
            entry.instructions.insert(idx + j, ins)


def _build_nc():
    import concourse.bass as bass
    import concourse.mybir as mybir
    import concourse.tile as tile
    from concourse import bacc

    _install_light_tail()

    f32 = mybir.dt.float32
    mm_dt = {
        "bf16": mybir.dt.bfloat16,
        "f32r": mybir.dt.float32r,
        "f32": f32,
    }[KERNEL_DT]
    _hoist = {"SP": [], "Pool": [], "Activation": []}
    # Bacc (not Bass): its compile() legalizes sem waits to <=1 per
    # instruction, which this walrus requires.
    nc = bacc.Bacc()
    x_d = nc.declare_dram_parameter("x", [S, FREE], mm_dt, isOutput=False)
    m_d = nc.declare_dram_parameter("mask", [S, K], f32, isOutput=False)
    o_d = nc.declare_dram_parameter("out", [S, FREE], mm_dt, isOutput=True)

    with tile.TileContext(nc) as tc:
        with (
            tc.tile_pool(name="singles", bufs=1) as singles,
            tc.tile_pool(name="xin", bufs=4) as xin,
            tc.tile_pool(name="oout", bufs=6) as oout,
            tc.tile_pool(name="psum", bufs=7, space="PSUM") as psum,
            tc.tile_pool(name="psumT", bufs=1, space="PSUM") as psumT,
        ):
            # ---- preload the Exp activation table off the critical path ----
            warm = singles.tile([S, 1], f32)
            nc.vector.memset(warm[:], 0.0)
            nc.scalar.activation(
                out=warm[:], in_=warm[:], func=mybir.ActivationFunctionType.Exp
            )

            # ---- shifted-identity bank: no input deps, starts immediately.
            # identW[p, g] = 1 iff g == p + PAD (128 x 148). The k-th shifted
            # identity D_k[p, f] = (f == p + k - PAD) is just the slice
            # identW[:, 2*PAD-k : 2*PAD-k+S] -- no per-k build needed.
            identW = singles.tile([S, S + 2 * PAD], f32)
            _hoist["Pool"].append(nc.gpsimd.memset(identW[:], 0.0).ins.name)
            nc.gpsimd.affine_select(
                out=identW[:],
                in_=identW[:],
                compare_op=mybir.AluOpType.not_equal,
                fill=1.0,
                base=PAD,
                # affine(p,g) = p - g + PAD ; == 0 -> fill 1.0
                pattern=[[-1, S + 2 * PAD]],
                channel_multiplier=1,
            )
            # identity in the matmul dtype (rhs of the band accumulation)
            identr = singles.tile([S, S], mm_dt)
            if KERNEL_DT == "bf16":
                _hoist["Pool"].append(
                    nc.gpsimd.memset(identr[:], 0.0).ins.name
                )
                nc.gpsimd.affine_select(
                    out=identr[:],
                    in_=identr[:],
                    compare_op=mybir.AluOpType.not_equal,
                    fill=1.0,
                    base=0,
                    pattern=[[-1, S]],
                    channel_multiplier=1,
                )
            else:
                # f32r tiles cannot be memset/selected directly; cast on DVE
                nc.vector.tensor_copy(
                    out=identr[:], in_=identW[:, PAD : PAD + S]
                )

            # ---- softmax numerator (normalization folded into epilogue) ----
            mask_t = singles.tile([S, K], f32)
            _hoist["SP"].append(
                nc.sync.dma_start(out=mask_t[:], in_=m_d[:]).ins.name
            )

            mx = singles.tile([S, 1], f32)
            nc.vector.reduce_max(mx[:], mask_t[:], axis=mybir.AxisListType.X)
            negmx = singles.tile([S, 1], f32)
            nc.vector.tensor_scalar_mul(negmx[:], mx[:], -1.0)

            wexp = singles.tile([S, K], f32)
            wsum = singles.tile([S, 1], f32)
            nc.scalar.activation(
                out=wexp[:],
                in_=mask_t[:],
                func=mybir.ActivationFunctionType.Exp,
                bias=negmx[:],
                scale=1.0,
                accum_out=wsum[:],
            )
            rsum = singles.tile([S, 1], f32)
            nc.vector.reciprocal(rsum[:], wsum[:])

            # ---- banded weight matrix ----
            # dw[p,f,k] = identW[p, f+2*PAD-k] * wexp[p,k] via ONE DVE mult
            # over a negative-k-stride view of identW, then a contiguous
            # k-reduce to E'[p,f] = wexp[p, f-p+PAD], then band = E'.T as a
            # single PE matmul against the identity.
            dw_all = singles.tile([S, S, K], f32)
            iw = identW[:]
            shifted = bass.AP(
                tensor=iw.tensor,
                offset=iw.offset + 2 * PAD,
                ap=[iw.ap[0], [1, S], [-1, K]],
            )
            nc.vector.tensor_tensor(
                dw_all[:],
                shifted,
                wexp[:, None, :].to_broadcast((S, S, K)),
                mybir.AluOpType.mult,
            )
            eprime = singles.tile([S, S], f32)
            nc.vector.reduce_sum(eprime[:], dw_all[:], axis=mybir.AxisListType.X)
            eprime_m = singles.tile([S, S], mm_dt)
            nc.vector.tensor_copy(out=eprime_m[:], in_=eprime[:])
            band_ps = psumT.tile([S, S], f32)
            nc.tensor.matmul(
                band_ps[:], lhsT=eprime_m[:], rhs=identr[:], start=True, stop=True
            )
            band = singles.tile([S, S], mm_dt)
            nc.vector.tensor_copy(out=band[:], in_=band_ps[:])

            # ---- stream x through the banded matmul ----
            # Issue ALL input DMAs before any output DMA so the SP issue
            # stream never blocks an input transfer behind an
            # epilogue-dependent output wait.
            n_chunks = FREE // CHUNK
            mm_per_chunk = CHUNK // MM_N
            xts = []
            for c in range(n_chunks):
                xt = xin.tile([S, CHUNK], mm_dt)
                _hoist["SP"].append(
                    nc.sync.dma_start(
                        out=xt[:], in_=x_d[:, c * CHUNK : (c + 1) * CHUNK]
                    ).ins.name
                )
                xts.append(xt)
            OUT_CHUNK = 1024
            n_out = FREE // OUT_CHUNK
            mm_per_out = OUT_CHUNK // MM_N
            for oc in range(n_out):
                xt = xts[(oc * OUT_CHUNK) // CHUNK]
                xbase = (oc * OUT_CHUNK) % CHUNK
                ot = oout.tile([S, OUT_CHUNK], mm_dt)
                for j in range(mm_per_out):
                    ps = psum.tile([S, MM_N], f32)
                    nc.tensor.matmul(
                        ps[:],
                        lhsT=band[:],
                        rhs=xt[:, xbase + j * MM_N : xbase + (j + 1) * MM_N],
                        start=True,
                        stop=True,
                    )
                    # epilogue: copy + softmax denominator (per-partition),
                    # alternating DVE / ScalarE to halve the epilogue wall
                    oslice = ot[:, j * MM_N : (j + 1) * MM_N]
                    if (oc * mm_per_out + j) % 2 == 0:
                        nc.vector.tensor_scalar_mul(oslice, ps[:], rsum[:])
                    else:
                        nc.scalar.activation(
                            out=oslice,
                            in_=ps[:],
                            func=mybir.ActivationFunctionType.Copy,
                            bias=0.0,
                            scale=rsum[:],
                        )
                nc.sync.dma_start(
                    out=o_d[:, oc * OUT_CHUNK : (oc + 1) * OUT_CHUNK],
                    in_=ot[:],
                )

    nc.finalize()
    _hoist_preamble(nc, _hoist)
    return nc


def _get_compiled():
    if "nc" not in _COMPILED:
        _COMPILED["nc"] = _build_nc()
    return _COMPILED["nc"]


def _rebuild_fallback():
    """Fallback: rebuild with the next-safer stream dtype."""
    global KERNEL_DT
    KERNEL_DT = {"bf16": "f32r", "f32r": "f32"}.get(KERNEL_DT, "f32")
    _COMPILED.pop("nc", None)
    return _get_compiled()


def _np_stream_dtype():
    import concourse.mybir as mybir

    return mybir.dt.np(
        {
            "bf16": mybir.dt.bfloat16,
            "f32r": mybir.dt.float32r,
            "f32": mybir.dt.float32,
        }[KERNEL_DT]
    )


def _shard_inputs(x, ada_mask):
    sdt = _np_stream_dtype()
    in_maps = []
    for i in range(N_CORES):
        b, h = divmod(i, H_SPLIT)
        xs = np.ascontiguousarray(
            x[b, :, h * HS : (h + 1) * HS, :].reshape(S, FREE)
        ).astype(sdt)
        ms = np.ascontiguousarray(ada_mask[b]).astype(np.float32, copy=False)
        in_maps.append({"x": xs, "mask": ms})
    return in_maps


def _run(x, ada_mask, trace=False, tmpdir=None):
    from concourse.bass_utils import run_bass_kernel_spmd

    res = None
    for attempt in range(4):
        nc = _get_compiled()
        in_maps = _shard_inputs(x, ada_mask)
        try:
            res = run_bass_kernel_spmd(
                nc,
                in_maps,
                core_ids=list(range(N_CORES)),
                trace=trace,
                tmpdir=tmpdir,
            )
            break
        except Exception:
            if attempt == 0:
                _COMPILED.pop("nc", None)  # transient: rebuild same dtype
            elif KERNEL_DT != "f32":
                _rebuild_fallback()        # step bf16 -> f32r -> f32
            else:
                raise
    assert res is not None
    out = np.empty((B, S, H, W), dtype=np.float32)
    for i in range(N_CORES):
        b, h = divmod(i, H_SPLIT)
        out[b, :, h * HS : (h + 1) * HS, :] = (
            res.results[i]["out"].astype(np.float32).reshape(S, HS, W)
        )
    return out, res


def kernel(x, ada_mask):
    x = np.asarray(x)
    ada_mask = np.asarray(ada_mask)
    out, _ = _run(x, ada_mask, trace=False)
    return out


def kernel_traced(x, ada_mask, tmpdir=None):
    """Correctness + profile run: returns (out, BassKernelResults)."""
    return _run(np.asarray(x), np.asarray(ada_mask), trace=True, tmpdir=tmpdir)
